# revision 42
# baseline (speedup 1.0000x reference)
"""CosArcLoss on 8 TRN2 NeuronCores (Bass/Tile), all-HWDGE fp8 pipeline.

Math (reference, f32):
    t_i   = preds[i, labels[i]]
    num_i = 30*(cos(arccos(clip(t_i)) + 0.5) - 0.35)
    S_i   = sum_{j != labels[i]} exp(30*preds[i,j])
    loss  = mean_i( log(exp(num_i) + S_i) - num_i )

Device does all O(B*V) work: sum_j exp(30*q(x_ij)) over fp8-quantized
inputs (tolerance 2e-2 >> fp8 logsumexp bias ~4e-3). Host does the O(B)
epilogue: numerator from exact f32 targets, subtraction of the (exactly
simulated) target-column device contribution, final log/mean.

Per-core layout (256 rows, 32000 classes), split by class:
  region A (classes [0, CA)):  row-major fp8 [row, class] tiles, HWDGE,
      ScalarE exp (scale=30) with free accum_out row-sums (~0.83 ns/col).
  region B (classes [CA, V)):  transposed fp8 [class, row] chunks of 128
      classes, plain HWDGE (fp8, no cast), VectorE computes exp via the
      Schraudolph exp2 bit-trick directly on fp8 input (2x_2p mode,
      ~0.52 ns/col; i16 = rint(x*S1+S2) whose bits ARE bf16(exp(30x))),
      TensorE ones-matmul reduces pairs of chunks (N=512 = one PSUM bank)
      into a single PSUM accumulator chain, evicted once at the end
      (DVE copy PSUM->SBUF, then DMA).
All data moves via HWDGE at full rate; no SWDGE cast (the old fp8->bf16
cast DMA serialized the kernel on the GpSimd sequencer). A/B DMA issue is
interleaved so both compute streams stay fed.
"""
import numpy as np
import ml_dtypes
from contextlib import ExitStack

import concourse.bass as bass
import concourse.tile as tile
from concourse import bacc, mybir
from concourse.bass_utils import run_bass_kernel_spmd

B, V = 2048, 32000
N_CORES = 8
RPC = B // N_CORES            # 256 rows per core
P = 128                       # SBUF partitions
G = RPC // P                  # 2 row groups (region A)

CA = 10752                    # classes handled by ACT (region A)
VB = V - CA                   # classes handled by DVE/GpSimd+TensorE (B)
NCH = VB // P                 # chunks of 128 classes (transposed)
ATILES = [1536, 3072, 3072, 2048, 1024]  # per-group ACT tiles (tapered)
NTA = len(ATILES)
assert sum(ATILES) == CA
DCHS = [4] + [10] * 16 + [2]  # chunks per B DMA (small first/last)
assert sum(DCHS) == NCH
VCH = 10                      # chunks per DVE schraudolph op
POOL_UNITS = {5, 10, 15}      # B units whose schraudolph runs on GpSimd
NBANK = 2                     # alternating PSUM banks (hides LDWEIGHTS)

SCALE = 30.0
LN2 = float(np.log(2.0))
S1 = 128.0 * SCALE / LN2           # schraudolph slope (bf16 bits / x)
C0 = 0.0564005                     # zero-mean-rel-err offset
S2 = 128.0 * (127.0 - C0)

F32 = mybir.dt.float32
BF16 = mybir.dt.bfloat16
I16 = mybir.dt.int16
FP8 = mybir.dt.float8e4
AF = mybir.ActivationFunctionType
ALU = mybir.AluOpType
E4M3 = ml_dtypes.float8_e4m3

_cache = {}


def _build():
    nc = bacc.Bacc("TRN2", target_bir_lowering=False, debug=False,
                   num_devices=N_CORES)
    xa = nc.dram_tensor("xa", [RPC, CA], FP8, kind="ExternalInput")
    xbt = nc.dram_tensor("xbt", [P, NCH * RPC], FP8, kind="ExternalInput")
    ident = nc.dram_tensor("ident", [P, P], F32, kind="ExternalInput")
    # osa is TRANSPOSED [G*NTA, 128]: a PE transpose turns the [128, G*NTA]
    # accum tensor into a few long rows, so the output DMA is 10 big
    # descriptors instead of 128 tiny ones.
    osa = nc.dram_tensor("osa", [G * NTA, P], F32, kind="ExternalOutput")
    osb = nc.dram_tensor("osb", [1, NBANK * 2 * RPC], F32,
                         kind="ExternalOutput")

    NMM = NCH // 2

    with tile.TileContext(nc) as tc, ExitStack() as ctx:
        apool = ctx.enter_context(tc.tile_pool(name="ap", bufs=5))
        bpool = ctx.enter_context(tc.tile_pool(name="bp", bufs=6))
        epool = ctx.enter_context(tc.tile_pool(name="ep", bufs=2))
        ipool = ctx.enter_context(tc.tile_pool(name="ip", bufs=4))
        spool = ctx.enter_context(tc.tile_pool(name="sp", bufs=1))
        psum = ctx.enter_context(tc.tile_pool(name="ps", bufs=1, space="PSUM"))

        ssum = spool.tile([P, G * NTA], F32)
        sb = spool.tile([1, NBANK * 2 * RPC], F32)
        sbT = spool.tile([G * NTA, P], F32)
        it = spool.tile([P, P], F32)
        nc.sync.dma_start(it[:], ident[:, :])
        ones = spool.tile([P, 1], BF16)
        nc.any.memset(ones[:], 1.0)
        banks = [psum.tile([P, 2 * RPC], F32, name=f"bank{k}")
                 for k in range(NBANK)]
        psT = psum.tile([P, P], F32, name="psT")

        MAXW = max(ATILES)
        MAXC = max(DCHS)
        nmm = 0

        # A unit: one fp8 tile (HWDGE) -> ACT exp + accum_out row sum.
        a_units = []
        aoff = 0
        for t in range(NTA):
            for g in range(G):
                a_units.append((g, t, aoff, ATILES[t]))
            aoff += ATILES[t]

        # B unit: one HWDGE fp8 DMA of k transposed chunks -> DVE
        # schraudolph (fp8 in, i16 out) -> k/2 paired matmuls into psum.
        b_units = []
        boff = 0
        for k in DCHS:
            b_units.append((boff, k))
            boff += k

        def emit_a(g, t, off, w):
            rs = slice(g * P, (g + 1) * P)
            xt = apool.tile([P, MAXW], FP8, tag="xt")
            nc.sync.dma_start(xt[:, :w], xa[rs, off:off + w])
            et = epool.tile([P, MAXW], BF16, tag="et")
            idx = g * NTA + t
            nc.scalar.activation(et[:, :w], xt[:, :w], AF.Exp, scale=SCALE,
                                 accum_out=ssum[:, idx:idx + 1])

        def mm(si, j0, j1):
            # paired matmuls over si chunk-pairs [j0, j1), alternating PSUM
            # banks so LDWEIGHTS of pair N+1 hides under pair N's matmul.
            nonlocal nmm
            for j in range(j0, j1):
                kb = nmm % NBANK
                rhs = si[:, j * 2 * RPC:(j + 1) * 2 * RPC].bitcast(BF16)
                nc.tensor.matmul(banks[kb][:1], ones[:], rhs,
                                 start=(nmm < NBANK),
                                 stop=(nmm >= NMM - NBANK),
                                 skip_group_check=True)
                nmm += 1

        def emit_b(ch0, k, on_pool=False):
            xb = bpool.tile([P, MAXC * RPC], FP8, tag="xb")
            nc.sync.dma_start(xb[:, :k * RPC],
                              xbt[:, ch0 * RPC:(ch0 + k) * RPC])
            si = ipool.tile([P, MAXC * RPC], I16, tag="si")
            if on_pool:
                # whole-unit schraudolph on GpSimd: one big op amortizes
                # the ~1.2us Q7 launch overhead and relieves DVE.
                nc.gpsimd.tensor_scalar(si[:, :k * RPC], xb[:, :k * RPC],
                                        S1, S2, ALU.mult, ALU.add)
                mm(si, 0, k // 2)
                return
            for v0 in range(0, k, VCH):
                v1 = min(v0 + VCH, k)
                nc.vector.tensor_scalar(si[:, v0 * RPC:v1 * RPC],
                                        xb[:, v0 * RPC:v1 * RPC],
                                        S1, S2, ALU.mult, ALU.add)
                mm(si, v0 // 2, v1 // 2)

        # Interleave A and B units by cumulative BYTES (weighted Bresenham,
        # A first): queue FIFO order then matches each stream's bandwidth
        # share, so neither engine waits behind the other's prefetch.
        abytes = sum(u[3] for u in a_units)
        bbytes = sum(u[1] * RPC for u in b_units)
        ia = ib = 0
        ca_b = cb_b = 0
        while ia < len(a_units) or ib < len(b_units):
            if ia < len(a_units) and (
                    ib >= len(b_units) or ca_b * bbytes <= cb_b * abytes):
                ca_b += a_units[ia][3]
                emit_a(*a_units[ia])
                ia += 1
            else:
                cb_b += b_units[ib][1] * RPC
                emit_b(*b_units[ib], on_pool=(ib in POOL_UNITS))
                ib += 1

        # outputs: transpose the accum tensor on the (now idle) PE so the
        # osa DMA is a handful of long descriptors; both output DMAs issue
        # from the Scalar sequencer, keeping Sync free for input issue.
        nc.tensor.transpose(psT[:G * NTA, :], ssum[:], it[:])
        nc.vector.tensor_copy(sbT[:, :], psT[:G * NTA, :])
        nc.scalar.dma_start(osa[:, :], sbT[:])
        W = 2 * RPC
        for kb in range(NBANK):
            nc.vector.tensor_copy(sb[:, kb * W:(kb + 1) * W], banks[kb][:1])
        nc.scalar.dma_start(osb[:, :], sb[:])

    nc.compile()
    return nc


def _get_nc():
    if "nc" not in _cache:
        _cache["nc"] = _build()
    return _cache["nc"]


def _shard(preds, labels):
    """Quantize to fp8-e4m3 and build per-core region A/B device layouts."""
    preds = np.ascontiguousarray(preds, dtype=np.float32)
    q = preds.astype(E4M3)
    eye = np.eye(P, dtype=np.float32)
    in_maps = []
    for c in range(N_CORES):
        rows = slice(c * RPC, (c + 1) * RPC)
        qa = np.ascontiguousarray(q[rows, :CA])
        # [256, VB] -> [VB, 256] -> chunks of 128 classes along free dim
        qb = np.ascontiguousarray(
            q[rows, CA:].T.reshape(NCH, P, RPC).transpose(1, 0, 2)
            .reshape(P, NCH * RPC))
        in_maps.append({"xa": qa, "xbt": qb, "ident": eye})
    return in_maps


def kernel(preds, labels):
    preds = np.ascontiguousarray(preds, dtype=np.float32)
    labels = np.asarray(labels).astype(np.int64)
    in_maps = _shard(preds, labels)
    nc = _get_nc()
    res = run_bass_kernel_spmd(nc, in_maps, list(range(N_CORES)))

    # device row sums S (all classes, fp8-quantized)
    S = np.empty(B, dtype=np.float64)
    for c in range(N_CORES):
        r = res.results[c]
        sa = np.asarray(r["osa"], np.float64)            # [G*NTA, 128]
        sb = np.asarray(r["osb"], np.float64)[0]         # [NBANK*512]
        s_a = np.zeros(RPC)
        for g in range(G):
            s_a[g * P:(g + 1) * P] = sa[g * NTA:(g + 1) * NTA, :].sum(axis=0)
        # each bank holds two 256-col half-sums (paired chunks)
        s_b = sb.reshape(NBANK * 2, RPC).sum(axis=0)
        S[c * RPC:(c + 1) * RPC] = s_a + s_b

    # subtract the device's own target-column contribution (exact simulation)
    idx = np.arange(B)
    tq32 = preds[idx, labels].astype(E4M3).astype(np.float32)
    in_a = labels < CA
    sub = np.empty(B, dtype=np.float64)
    sub[in_a] = np.exp(np.float64(SCALE) * tq32[in_a].astype(np.float64))
    vb = (tq32[~in_a] * np.float32(S1) + np.float32(S2)).astype(np.float32)
    i16 = np.rint(vb.astype(np.float64)).astype(np.int16)
    sub[~in_a] = i16.view(ml_dtypes.bfloat16).astype(np.float64)
    S_others = S - sub

    # numerator from exact f32 targets (reference formula)
    t = preds[idx, labels].astype(np.float64)
    eps = 1e-12
    theta = np.arccos(np.clip(t, -1.0 + eps, 1.0 - eps))
    theta = np.clip(theta, eps, np.pi - eps)
    num = SCALE * (np.cos(theta + 0.5) - 0.35)

    den = np.exp(num) + S_others
    loss = np.mean(np.log(den) - num)
    return np.array(loss, dtype=np.float32)
